# revision 22
# baseline (speedup 1.0000x reference)
"""Bilateral filter denoiser (5x5, sigma_s=2.0, sigma_r=0.1) on 8 Trainium2
NeuronCores.  Takes full inputs x (8,3,512,512) f32 + spatial (5,5) f32;
pure data parallel: one batch element per core; returns the full output.

Per-core kernel (Bass/Tile), symmetric half-offset formulation:
  For each of the 12 half offsets t=(di,dj):
      Wraw_t[g] = (2/sqrt(pi)) * exp(-50*(xp[g+t]-xp[g])^2)   [Derivative_Erf]
      mraw_t    = Wraw_t * (xp[g+t]-xp[g])
  With a_t = s_t*sqrt(pi)/2 folded into the matmul stationaries:
      S0 = s_c + sum_t a_t*(Wraw_t[g] + Wraw_t[g-t])
      S1 = sum_t a_t*(mraw_t[g] - mraw_t[g-t])
      out = x + S1/S0        (S0 >= s_c > 0: the 1e-10 clip never binds)
  Reflect padding makes the shared-weight trick exact at image borders.

Engine assignment (four-way balance DVE/ACT/Pool/PE):
  * dt on DVE (fp16 tensor_tensor, 2x mode).  W on ACT via ONE
    Derivative_Erf per offset (squaring is inside the table function;
    scale=sqrt(50) via ACT's free affine) -- no Square ops anywhere.
  * 'P' offsets (|dj|=2 ring, 5): W fp8e4 from ACT; m = W*dt on GpSimd
    in fp8; S0 and S1 each accumulate via ONE fp8 DoubleRow matmul per
    strip (2 k-tiles = fw+bw taps at col-stride |dj|, 0.5 cycles/row).
    NB k-tile stride 1 hangs the PE -- only |dj|=2 offsets can DR.
  * 'Q' offset (2,1): W+m fp8 as above but two plain fp8 matmuls each.
  * 'D'/'D0' offsets: W fp16, m on DVE fp16 (2x); fp16 matmuls (dj==0
    uses combined fw+bw matrices).
  * epilogue: S0 evacuated on ACT (Identity, +s_c bias -- replaces a
    center-tap matmul), fast-reciprocal + S1*Rc + (+x) on DVE fp16;
    y stored fp16 (host casts to fp32).
  * 13 strips of 128 padded rows in groups of 3 (+1 warmup group); PSUM
    tags rotate by global strip index so consecutive groups land in banks
    freed a group earlier (decouples the boundary drain chain and keeps
    the PE p-state ramped); field ops cover a whole group via 2-level APs;
    m+matmul emission is software-pipelined one offset behind dt/derf.
"""

import numpy as np

B, C, H, W = 8, 3, 512, 512
SQRT50 = 7.0710678118654755
DERF_SCALE = 0.8862269254527579  # sqrt(pi)/2: Derivative_Erf = 2/sqrt(pi)*exp(-x^2)

# (di, dj, cls). cls: 'P' fp8 DoubleRow, 'Q' fp8 plain, 'D' fp16, 'D0' dj==0.
# Pool computes m for P/Q (6 offsets), DVE for D/D0 (6).  fp8 m lands on the
# smallest spatial weights.  Order interleaves Pool-m and DVE-m classes.
OFFSETS = [
    (0, 1, 'D'), (1, -2, 'P'), (1, -1, 'D'), (2, 2, 'P'),
    (1, 0, 'D0'), (1, 2, 'P'), (1, 1, 'D'), (2, -2, 'P'),
    (2, 1, 'Q'), (2, 0, 'D0'), (0, 2, 'P'), (2, -1, 'D'),
]
GSZ = 3

_CACHE = {}


def _strip_plan():
    Hp = H + 4
    R = C * Hp
    strips = []
    rbase = 0
    while R - 4 - rbase > 0:
        strips.append((rbase, min(124, R - 4 - rbase)))
        rbase += 124
    return strips[-1][0] + 132, strips


# index of each fp8-class (P/Q) offset into the sh8 stationary block and of
# each D-class offset into the sh16 block
OFF8_INDEX = {}
OFFD_INDEX = {}
for _d in OFFSETS:
    if _d[2] in ('P', 'Q'):
        OFF8_INDEX[(_d[0], _d[1])] = len(OFF8_INDEX)
    elif _d[2] == 'D':
        OFFD_INDEX[(_d[0], _d[1])] = len(OFFD_INDEX)


def _build():
    from contextlib import ExitStack

    import concourse.bacc as bacc
    import concourse.bass as bass
    import concourse.tile as tile
    from concourse import mybir

    F32 = mybir.dt.float32
    F16 = mybir.dt.float16
    F8 = mybir.dt.float8e4
    Alu = mybir.AluOpType
    Act = mybir.ActivationFunctionType
    DR = mybir.MatmulPerfMode.DoubleRow

    Hp, Wp = H + 4, W + 4
    R = C * Hp
    Rpad, strips = _strip_plan()
    NS = Wp  # per-strip slot width inside group tiles

    n8 = len(OFF8_INDEX)
    nD = len(OFFD_INDEX)
    nD0 = sum(1 for o in OFFSETS if o[2] == 'D0')

    def pairap(v, col0, nh, wd):
        return bass.AP(tensor=v.tensor, offset=v.offset + col0,
                       ap=[v.ap[0], [NS, nh], [1, wd]])

    def drap(v, col0, stride, wd):
        # 2-k-tile moving operand: windows at col0 and col0+stride
        return bass.AP(tensor=v.tensor, offset=v.offset + col0,
                       ap=[v.ap[0], [stride, 2], [1, wd]])

    def statpair(v):
        return bass.AP(tensor=v.tensor, offset=v.offset,
                       ap=[v.ap[0], [128, 2], [1, 128]])

    nc = bacc.Bacc(
        "TRN2",
        target_bir_lowering=False,
        debug=False,
        enable_asserts=False,
        num_devices=B,
    )
    xp = nc.dram_tensor("xp", [Rpad, Wp], F16, kind="ExternalInput").ap()
    # stationaries packed host-side as [128, N] so each loads in ONE DMA
    # (28 separate loads serialized ~650ns each on the SP sequencer and
    # delayed kernel start by ~26us)
    sh8 = nc.dram_tensor("sh8", [128, 2 * n8 * 256], F8,
                         kind="ExternalInput").ap()
    sh16 = nc.dram_tensor("sh16", [128, (3 * nD + 2 * nD0) * 128], F16,
                          kind="ExternalInput").ap()
    scv = nc.dram_tensor("scv", [1, 1], F32, kind="ExternalInput").ap()
    y = nc.dram_tensor("y", [C * H, W], F16, kind="ExternalOutput").ap()

    with tile.TileContext(nc) as tc, ExitStack() as ctx:
        consts = ctx.enter_context(tc.tile_pool(name="consts", bufs=1))
        # consts ride the Pool (SWDGE) queue so the SP queue dispatches the
        # first slab loads immediately
        SH8 = consts.tile([128, 2 * n8 * 256], F8, name="SH8", tag="SH8")
        SH16 = consts.tile([128, (3 * nD + 2 * nD0) * 128], F16,
                           name="SH16", tag="SH16")
        M8 = [SH8[:, s * 256:(s + 1) * 256] for s in range(2 * n8)]
        M16 = [SH16[:, s * 128:(s + 1) * 128]
               for s in range(3 * nD + 2 * nD0)]
        sc = consts.tile([128, 1], F32)
        nc.gpsimd.dma_start(out=sc[:], in_=scv.to_broadcast([128, 1]))

        slabs = ctx.enter_context(tc.tile_pool(name="slabs", bufs=3))
        fld = ctx.enter_context(tc.tile_pool(name="fld", bufs=7))
        accp = ctx.enter_context(tc.tile_pool(name="accum", bufs=5))
        psum = ctx.enter_context(tc.tile_pool(name="psum", bufs=1, space="PSUM"))

        # tiny group FIRST: it pipeline-fills while the startup DMAs are
        # still serializing, and the kernel then ends on a wide group
        seq = list(enumerate(strips))
        seq = seq[-1:] + seq[:-1]
        seq = [(pos, rk[1]) for pos, rk in enumerate(seq)]
        groups = [seq[:1]] + [seq[1 + i:1 + i + GSZ]
                              for i in range(0, len(seq) - 1, GSZ)]

        def load_slab(grp, T, v):
            nh = len(grp)
            rb0 = grp[0][1][0]
            # one DMA per slab: strips advance by 124 rows in DRAM and
            # by NS cols in SBUF
            src_ap = bass.AP(
                tensor=xp.tensor,
                offset=xp.offset + (rb0 + v) * Wp,
                ap=[[Wp, 128], [124 * Wp, nh], [1, Wp]])
            nc.sync.dma_start(out=pairap(T[v][:, :], 0, nh, Wp), in_=src_ap)

        def load_slabs(grp):
            nh = len(grp)
            T = [slabs.tile([128, nh * NS], F16, tag=f"T{v}", name=f"T{v}")
                 for v in range(3)]
            for v in range(3):
                load_slab(grp, T, v)
            return T

        # Only T0 of group 0 loads up front: the in-order DMA queue means a
        # consumer waits for every EARLIER-emitted DMA, so all other loads
        # are emitted after the first dt/derf (see the offset loop)
        grp0 = groups[0]
        T_next = [slabs.tile([128, len(grp0) * NS], F16, tag=f"T{v}",
                             name=f"T{v}") for v in range(3)]
        load_slab(grp0, T_next, 0)

        def emit_epilogue_strip(grp, T, S0, S1, h, tail=False):
                (rbase, K) = grp[h][1]
                # S0 evacuation on ACT folds in the center tap via bias
                S0s = accp.tile([128, W], F32, tag="S0s", name="S0s")
                nc.scalar.activation(S0s[:K, :], S0[h][:K, :], Act.Identity,
                                     bias=sc[:K, :])
                Rc = accp.tile([128, W], F32, tag="Rc", name="Rc")
                nc.vector.reciprocal_approx_fast(out=Rc[:K, :],
                                                 in_=S0s[:K, :])
                tmp = accp.tile([128, W], F16, tag="tmp", name="tmp")
                nc.vector.tensor_tensor(
                    out=tmp[:K, :], in0=S1[h][:K, :], in1=Rc[:K, :],
                    op=Alu.mult)
                res = accp.tile([128, W], F16, tag="res", name="res")
                nc.vector.tensor_tensor(
                    out=res[:K, :], in0=tmp[:K, :],
                    in1=T[2][0:K, h * NS + 2:h * NS + 2 + W], op=Alu.add)

                k = 0
                while k < K:
                    g = rbase + 2 + k
                    if g < R and 2 <= (g % Hp) <= Hp - 3:
                        k1 = k
                        while k1 < K:
                            g1 = rbase + 2 + k1
                            if g1 >= R or not (2 <= (g1 % Hp) <= Hp - 3):
                                break
                            if (g1 % Hp) == 2 and k1 > k:
                                break
                            k1 += 1
                        h0 = (g // Hp) * H + (g % Hp) - 2
                        nc.scalar.dma_start(out=y[h0:h0 + (k1 - k), :],
                                            in_=res[k:k1, :])
                        k = k1
                    else:
                        k += 1

        pending = None
        for gi, grp in enumerate(groups):
            nh = len(grp)
            T = T_next

            # PSUM tags rotate by GLOBAL strip index so a new group's first
            # strips land in banks freed >=1 group ago (partial decoupling
            # of the boundary drain chain)
            S0 = [psum.tile([128, W], F32, tag=f"S0p{si % 4}",
                            name=f"S0p{si % 4}") for si, _ in grp]
            S1 = [psum.tile([128, W], F32, tag=f"S1p{si % 4}",
                            name=f"S1p{si % 4}") for si, _ in grp]

            def emit_mm(oi, dt, wfld):
                # m + matmuls for offset oi, deferred one offset so the
                # in-order DVE/Pool queues never head-of-line block on the
                # ACT derf round trip
                di, dj, cls = OFFSETS[oi]
                cl = min(0, -dj)
                wd = W + abs(dj)
                first = oi == 0
                last = oi == len(OFFSETS) - 1
                fw = 2
                bw = 2 - dj
                if cls in ('P', 'Q'):
                    w8 = wfld
                    m8 = fld.tile([128, nh * NS], F8, tag="m8", name="m8", bufs=8)
                    nc.gpsimd.tensor_tensor(
                        out=pairap(m8[:, :], cl + 2, nh, wd),
                        in0=pairap(w8[:, :], cl + 2, nh, wd),
                        in1=pairap(dt[:, :], cl + 2, nh, wd),
                        op=Alu.mult)
                    idx = 2 * OFF8_INDEX[(di, dj)]
                    st0, st1 = M8[idx], M8[idx + 1]
                    if cls == 'P':
                        w0 = min(fw, bw)
                        stride = abs(dj)
                        for h in range(nh):
                            o = h * NS
                            nc.tensor.matmul(
                                S0[h][:, :], statpair(st0),
                                drap(w8[:, :], o + w0, stride, W),
                                start=first, stop=last, perf_mode=DR)
                            nc.tensor.matmul(
                                S1[h][:, :], statpair(st1),
                                drap(m8[:, :], o + w0, stride, W),
                                start=first, stop=last, perf_mode=DR)
                    else:  # Q: two plain fp8 matmuls each, [fw|bw] layout
                        def half(v, c0):
                            return bass.AP(tensor=v.tensor,
                                           offset=v.offset + c0,
                                           ap=[v.ap[0], [1, 128]])
                        for h in range(nh):
                            o = h * NS
                            nc.tensor.matmul(S0[h][:, :], half(st0, 0),
                                             w8[:, o + fw:o + fw + W],
                                             start=first, stop=False)
                            nc.tensor.matmul(S0[h][:, :], half(st0, 128),
                                             w8[:, o + bw:o + bw + W],
                                             start=False, stop=last)
                            nc.tensor.matmul(S1[h][:, :], half(st1, 0),
                                             m8[:, o + fw:o + fw + W],
                                             start=first, stop=False)
                            nc.tensor.matmul(S1[h][:, :], half(st1, 128),
                                             m8[:, o + bw:o + bw + W],
                                             start=False, stop=last)
                else:
                    w16 = wfld
                    m16 = fld.tile([128, nh * NS], F16, tag="m16", name="m16", bufs=8)
                    nc.vector.tensor_tensor(
                        out=pairap(m16[:, :], cl + 2, nh, wd),
                        in0=pairap(w16[:, :], cl + 2, nh, wd),
                        in1=pairap(dt[:, :], cl + 2, nh, wd),
                        op=Alu.mult)
                    if cls == 'D0':
                        base = 3 * nD + 2 * (0 if di == 1 else 1)
                        c0, c1 = M16[base], M16[base + 1]
                        for h in range(nh):
                            o = h * NS
                            nc.tensor.matmul(S0[h][:, :], c0,
                                             w16[:, o + fw:o + fw + W],
                                             start=first, stop=last)
                            nc.tensor.matmul(S1[h][:, :], c1,
                                             m16[:, o + fw:o + fw + W],
                                             start=first, stop=last)
                    else:
                        # [a*E2, a*E_{2-di}, -a*E_{2-di}]
                        base = 3 * OFFD_INDEX[(di, dj)]
                        afw, abw, nbw = (M16[base], M16[base + 1],
                                         M16[base + 2])
                        for h in range(nh):
                            o = h * NS
                            nc.tensor.matmul(S0[h][:, :], afw,
                                             w16[:, o + fw:o + fw + W],
                                             start=first, stop=False)
                            nc.tensor.matmul(S0[h][:, :], abw,
                                             w16[:, o + bw:o + bw + W],
                                             start=False, stop=last)
                            nc.tensor.matmul(S1[h][:, :], afw,
                                             m16[:, o + fw:o + fw + W],
                                             start=first, stop=False)
                            nc.tensor.matmul(S1[h][:, :], nbw,
                                             m16[:, o + bw:o + bw + W],
                                             start=False, stop=last)

            deferred = None
            ep_h = 0
            for oi, (di, dj, cls) in enumerate(OFFSETS):
                cl = min(0, -dj)
                wd = W + abs(dj)

                dt = fld.tile([128, nh * NS], F16, tag="dt", name="dt",
                              bufs=10)
                nc.vector.tensor_tensor(
                    out=pairap(dt[:, :], cl + 2, nh, wd),
                    in0=pairap(T[di][:, :], cl + dj + 2, nh, wd),
                    in1=pairap(T[0][:, :], cl + 2, nh, wd),
                    op=Alu.subtract)
                wfld = fld.tile([128, nh * NS],
                                F8 if cls in ('P', 'Q') else F16,
                                tag="w8" if cls in ('P', 'Q') else "w16",
                                name="w", bufs=8)
                nc.scalar.activation(
                    pairap(wfld[:, :], cl + 2, nh, wd),
                    pairap(dt[:, :], cl + 2, nh, wd),
                    Act.Derivative_Erf, bias=0.0, scale=SQRT50)

                if deferred is not None:
                    emit_mm(*deferred)
                deferred = (oi, dt, wfld)

                if oi == 0 and gi == 0:
                    # rest of the startup loads, after the first dt/derf
                    nc.sync.dma_start(out=SH16[:, :], in_=sh16)
                    nc.sync.dma_start(out=SH8[:, :], in_=sh8)
                    load_slab(grp, T, 1)
                    load_slab(grp, T, 2)
                if oi == 1 and gi + 1 < len(groups):
                    # prefetch next group's slabs AFTER this group's first dts
                    T_next = load_slabs(groups[gi + 1])

                # previous group's epilogue: one strip per offset slot so the
                # PSUM-drain chain spreads through the ACT/DVE queues
                if oi >= 5 and pending is not None:
                    emit_epilogue_strip(*pending, ep_h)
                    ep_h += 1
                    if ep_h == len(pending[0]):
                        pending = None
                        ep_h = 0
            emit_mm(*deferred)

            pending = (grp, T, S0, S1)
        for h in range(len(pending[0])):
            emit_epilogue_strip(*pending, h, tail=True)

    nc.compile()
    return nc


def _get_module():
    if "nc" not in _CACHE:
        _CACHE["nc"] = _build()
    return _CACHE["nc"]


def _pack_core(xc):
    """xc [C,H,W] f32 -> reflect-padded fp16 [Rpad, W+4]."""
    Rpad, _ = _strip_plan()
    xpad = np.pad(xc, ((0, 0), (2, 2), (2, 2)), mode="reflect")
    flat = xpad.reshape(C * (H + 4), W + 4)
    extra = Rpad - flat.shape[0]
    if extra > 0:
        flat = np.concatenate([flat, np.repeat(flat[-1:], extra, axis=0)],
                              axis=0)
    return np.ascontiguousarray(flat, dtype=np.float16)


def _stationaries(spatial):
    import ml_dtypes

    def E(q):  # lhsT=eye(k=-q) => out[r] = moving[r+q]
        return np.eye(128, 128, k=-q, dtype=np.float32)

    sh8_list = [None] * (2 * len(OFF8_INDEX))
    nD = len(OFFD_INDEX)
    sh16_list = [None] * (3 * nD + 4)
    for di, dj, cls in OFFSETS:
        a = float(spatial[2 + di, 2 + dj]) * DERF_SCALE
        fwm = a * E(2)
        if cls in ('P', 'Q'):
            bwm = a * E(2 - di)
            if cls == 'P' and dj > 0:  # bw window (2-dj) comes first
                s0p = np.concatenate([bwm, fwm], axis=1)
                s1p = np.concatenate([-bwm, fwm], axis=1)
            else:                      # [fw|bw] (all Q, and P with dj<0)
                s0p = np.concatenate([fwm, bwm], axis=1)
                s1p = np.concatenate([fwm, -bwm], axis=1)
            idx = 2 * OFF8_INDEX[(di, dj)]
            sh8_list[idx] = s0p
            sh8_list[idx + 1] = s1p
        elif cls == 'D0':
            base = 3 * nD + 2 * (0 if di == 1 else 1)
            sh16_list[base] = a * (E(2) + E(2 - di))
            sh16_list[base + 1] = a * (E(2) - E(2 - di))
        else:
            base = 3 * OFFD_INDEX[(di, dj)]
            sh16_list[base] = fwm
            sh16_list[base + 1] = a * E(2 - di)
            sh16_list[base + 2] = -a * E(2 - di)
    sh8 = np.concatenate(sh8_list, axis=1).astype(ml_dtypes.float8_e4m3)
    sh16 = np.concatenate(sh16_list, axis=1).astype(np.float16)
    scv = np.array([[spatial[2, 2]]], dtype=np.float32)
    return sh8, sh16, scv


def kernel(x, spatial, _trace=False):
    from concourse.bass_utils import run_bass_kernel_spmd

    x = np.asarray(x, dtype=np.float32)
    spatial = np.asarray(spatial, dtype=np.float32)
    assert x.shape == (B, C, H, W) and spatial.shape == (5, 5)
    # weight-field sharing between forward/backward taps needs symmetry
    assert np.allclose(spatial, spatial[::-1, ::-1], rtol=1e-5), \
        "kernel assumes point-symmetric spatial weights"

    sh8, sh16, scv = _stationaries(spatial)
    nc = _get_module()
    in_maps = [{"xp": _pack_core(x[b]), "sh8": sh8, "sh16": sh16, "scv": scv}
               for b in range(B)]
    res = run_bass_kernel_spmd(nc, in_maps, core_ids=list(range(B)),
                               trace=_trace)
    out = np.stack([res.results[b]["y"].astype(np.float32).reshape(C, H, W)
                    for b in range(B)])
    if _trace:
        return out, res
    return out


# revision 23
# speedup vs baseline: 1.0554x; 1.0554x over previous
"""Bilateral filter denoiser (5x5, sigma_s=2.0, sigma_r=0.1) on 8 Trainium2
NeuronCores.  Takes full inputs x (8,3,512,512) f32 + spatial (5,5) f32;
pure data parallel: one batch element per core; returns the full output.

Per-core kernel (Bass/Tile), symmetric half-offset formulation:
  For each of the 12 half offsets t=(di,dj):
      Wraw_t[g] = (2/sqrt(pi)) * exp(-50*(xp[g+t]-xp[g])^2)   [Derivative_Erf]
      mraw_t    = Wraw_t * (xp[g+t]-xp[g])
  With a_t = s_t*sqrt(pi)/2 folded into the matmul stationaries:
      S0 = s_c + sum_t a_t*(Wraw_t[g] + Wraw_t[g-t])
      S1 = sum_t a_t*(mraw_t[g] - mraw_t[g-t])
      out = x + S1/S0        (S0 >= s_c > 0: the 1e-10 clip never binds)
  Reflect padding makes the shared-weight trick exact at image borders.

Engine assignment (four-way balance DVE/ACT/Pool/PE):
  * dt on DVE (fp16 tensor_tensor, 2x mode).  W on ACT via ONE
    Derivative_Erf per offset (squaring is inside the table function;
    scale=sqrt(50) via ACT's free affine) -- no Square ops anywhere.
  * 'P' offsets (|dj|=2 ring, 5): W fp8e4 from ACT; m = W*dt on GpSimd
    in fp8; S0 and S1 each accumulate via ONE fp8 DoubleRow matmul per
    strip (2 k-tiles = fw+bw taps at col-stride |dj|, 0.5 cycles/row).
    NB k-tile stride 1 hangs the PE -- only |dj|=2 offsets can DR.
  * 'Q' offset (2,1): W+m fp8 as above but two plain fp8 matmuls each.
  * 'D'/'D0' offsets: W fp16, m on DVE fp16 (2x); fp16 matmuls (dj==0
    uses combined fw+bw matrices).
  * epilogue: S0 evacuated on ACT (Identity, +s_c bias -- replaces a
    center-tap matmul), fast-reciprocal + S1*Rc + (+x) on DVE fp16;
    y stored fp16 (host casts to fp32).
  * 13 strips of 128 padded rows in groups of 3 (+1 warmup group); PSUM
    tags rotate by global strip index so consecutive groups land in banks
    freed a group earlier (decouples the boundary drain chain and keeps
    the PE p-state ramped); field ops cover a whole group via 2-level APs;
    m+matmul emission is software-pipelined one offset behind dt/derf.
"""

import numpy as np

B, C, H, W = 8, 3, 512, 512
SQRT50 = 7.0710678118654755
DERF_SCALE = 0.8862269254527579  # sqrt(pi)/2: Derivative_Erf = 2/sqrt(pi)*exp(-x^2)

# (di, dj, cls). cls: 'P' fp8 DoubleRow, 'Q' fp8 plain, 'D' fp16, 'D0' dj==0.
# Pool computes m for P/Q (6 offsets), DVE for D/D0 (6).  fp8 m lands on the
# smallest spatial weights.  Order interleaves Pool-m and DVE-m classes.
OFFSETS = [
    (0, 1, 'D'), (1, -2, 'P'), (1, -1, 'D'), (2, 2, 'P'),
    (1, 0, 'D0'), (1, 2, 'P'), (1, 1, 'D'), (2, -2, 'P'),
    (2, 1, 'Q'), (2, 0, 'D0'), (0, 2, 'P'), (2, -1, 'D'),
]
GSZ = 3

_CACHE = {}


def _strip_plan():
    Hp = H + 4
    R = C * Hp
    strips = []
    rbase = 0
    while R - 4 - rbase > 0:
        strips.append((rbase, min(124, R - 4 - rbase)))
        rbase += 124
    return strips[-1][0] + 132, strips


# index of each fp8-class (P/Q) offset into the sh8 stationary block and of
# each D-class offset into the sh16 block
OFF8_INDEX = {}
OFFD_INDEX = {}
for _d in OFFSETS:
    if _d[2] in ('P', 'Q'):
        OFF8_INDEX[(_d[0], _d[1])] = len(OFF8_INDEX)
    elif _d[2] == 'D':
        OFFD_INDEX[(_d[0], _d[1])] = len(OFFD_INDEX)


def _build():
    from contextlib import ExitStack

    import concourse.bacc as bacc
    import concourse.bass as bass
    import concourse.tile as tile
    from concourse import mybir

    F32 = mybir.dt.float32
    F16 = mybir.dt.float16
    F8 = mybir.dt.float8e4
    Alu = mybir.AluOpType
    Act = mybir.ActivationFunctionType
    DR = mybir.MatmulPerfMode.DoubleRow

    Hp, Wp = H + 4, W + 4
    R = C * Hp
    Rpad, strips = _strip_plan()
    NS = Wp  # per-strip slot width inside group tiles

    n8 = len(OFF8_INDEX)
    nD = len(OFFD_INDEX)
    nD0 = sum(1 for o in OFFSETS if o[2] == 'D0')

    def pairap(v, col0, nh, wd):
        return bass.AP(tensor=v.tensor, offset=v.offset + col0,
                       ap=[v.ap[0], [NS, nh], [1, wd]])

    def drap(v, col0, stride, wd):
        # 2-k-tile moving operand: windows at col0 and col0+stride
        return bass.AP(tensor=v.tensor, offset=v.offset + col0,
                       ap=[v.ap[0], [stride, 2], [1, wd]])

    def statpair(v):
        return bass.AP(tensor=v.tensor, offset=v.offset,
                       ap=[v.ap[0], [128, 2], [1, 128]])

    nc = bacc.Bacc(
        "TRN2",
        target_bir_lowering=False,
        debug=False,
        enable_asserts=False,
        num_devices=B,
    )
    xp = nc.dram_tensor("xp", [Rpad, Wp], F16, kind="ExternalInput").ap()
    # stationaries packed host-side as [128, N] so each loads in ONE DMA
    # (28 separate loads serialized ~650ns each on the SP sequencer and
    # delayed kernel start by ~26us)
    sh8 = nc.dram_tensor("sh8", [128, 2 * n8 * 256], F8,
                         kind="ExternalInput").ap()
    sh16 = nc.dram_tensor("sh16", [128, (3 * nD + 2 * nD0) * 128], F16,
                          kind="ExternalInput").ap()
    scv = nc.dram_tensor("scv", [1, 1], F32, kind="ExternalInput").ap()
    y = nc.dram_tensor("y", [C * H, W], F16, kind="ExternalOutput").ap()

    with tile.TileContext(nc) as tc, ExitStack() as ctx:
        consts = ctx.enter_context(tc.tile_pool(name="consts", bufs=1))
        # consts ride the Pool (SWDGE) queue so the SP queue dispatches the
        # first slab loads immediately
        SH8 = consts.tile([128, 2 * n8 * 256], F8, name="SH8", tag="SH8")
        SH16 = consts.tile([128, (3 * nD + 2 * nD0) * 128], F16,
                           name="SH16", tag="SH16")
        M8 = [SH8[:, s * 256:(s + 1) * 256] for s in range(2 * n8)]
        M16 = [SH16[:, s * 128:(s + 1) * 128]
               for s in range(3 * nD + 2 * nD0)]
        sc = consts.tile([128, 1], F32)
        nc.gpsimd.dma_start(out=sc[:], in_=scv.to_broadcast([128, 1]))

        slabs = ctx.enter_context(tc.tile_pool(name="slabs", bufs=3))
        fld = ctx.enter_context(tc.tile_pool(name="fld", bufs=7))
        accp = ctx.enter_context(tc.tile_pool(name="accum", bufs=5))
        psum = ctx.enter_context(tc.tile_pool(name="psum", bufs=1, space="PSUM"))

        # tiny group FIRST: it pipeline-fills while the startup DMAs are
        # still serializing, and the kernel then ends on a wide group
        seq = list(enumerate(strips))
        seq = seq[-1:] + seq[:-1]
        seq = [(pos, rk[1]) for pos, rk in enumerate(seq)]
        groups = [seq[:1]] + [seq[1 + i:1 + i + GSZ]
                              for i in range(0, len(seq) - 1, GSZ)]

        def load_slab(grp, T, v):
            nh = len(grp)
            rb0 = grp[0][1][0]
            # one DMA per slab: strips advance by 124 rows in DRAM and
            # by NS cols in SBUF
            src_ap = bass.AP(
                tensor=xp.tensor,
                offset=xp.offset + (rb0 + v) * Wp,
                ap=[[Wp, 128], [124 * Wp, nh], [1, Wp]])
            nc.sync.dma_start(out=pairap(T[v][:, :], 0, nh, Wp), in_=src_ap)

        def load_slabs(grp):
            nh = len(grp)
            T = [slabs.tile([128, nh * NS], F16, tag=f"T{v}", name=f"T{v}")
                 for v in range(3)]
            for v in range(3):
                load_slab(grp, T, v)
            return T

        # Only T0 of group 0 loads up front: the in-order DMA queue means a
        # consumer waits for every EARLIER-emitted DMA, so all other loads
        # are emitted after the first dt/derf (see the offset loop)
        grp0 = groups[0]
        T_next = [slabs.tile([128, len(grp0) * NS], F16, tag=f"T{v}",
                             name=f"T{v}") for v in range(3)]
        load_slab(grp0, T_next, 0)

        def emit_epilogue_strip(grp, T, S0, S1, h, tail=False):
                (rbase, K) = grp[h][1]
                # S0 evacuation on ACT folds in the center tap via bias
                S0s = accp.tile([128, W], F32, tag="S0s", name="S0s")
                nc.scalar.activation(S0s[:K, :], S0[h][:K, :], Act.Identity,
                                     bias=sc[:K, :])
                Rc = accp.tile([128, W], F32, tag="Rc", name="Rc")
                nc.vector.reciprocal_approx_fast(out=Rc[:K, :],
                                                 in_=S0s[:K, :])
                tmp = accp.tile([128, W], F16, tag="tmp", name="tmp")
                nc.vector.tensor_tensor(
                    out=tmp[:K, :], in0=S1[h][:K, :], in1=Rc[:K, :],
                    op=Alu.mult)
                res = accp.tile([128, W], F16, tag="res", name="res")
                nc.vector.tensor_tensor(
                    out=res[:K, :], in0=tmp[:K, :],
                    in1=T[2][0:K, h * NS + 2:h * NS + 2 + W], op=Alu.add)

                k = 0
                while k < K:
                    g = rbase + 2 + k
                    if g < R and 2 <= (g % Hp) <= Hp - 3:
                        k1 = k
                        while k1 < K:
                            g1 = rbase + 2 + k1
                            if g1 >= R or not (2 <= (g1 % Hp) <= Hp - 3):
                                break
                            if (g1 % Hp) == 2 and k1 > k:
                                break
                            k1 += 1
                        h0 = (g // Hp) * H + (g % Hp) - 2
                        nc.sync.dma_start(out=y[h0:h0 + (k1 - k), :],
                                            in_=res[k:k1, :])
                        k = k1
                    else:
                        k += 1

        pending = None
        for gi, grp in enumerate(groups):
            nh = len(grp)
            T = T_next

            # PSUM tags rotate by GLOBAL strip index so a new group's first
            # strips land in banks freed >=1 group ago (partial decoupling
            # of the boundary drain chain)
            S0 = [psum.tile([128, W], F32, tag=f"S0p{si % 4}",
                            name=f"S0p{si % 4}") for si, _ in grp]
            S1 = [psum.tile([128, W], F32, tag=f"S1p{si % 4}",
                            name=f"S1p{si % 4}") for si, _ in grp]

            def emit_mm(oi, dt, wfld):
                # m + matmuls for offset oi, deferred one offset so the
                # in-order DVE/Pool queues never head-of-line block on the
                # ACT derf round trip
                di, dj, cls = OFFSETS[oi]
                cl = min(0, -dj)
                wd = W + abs(dj)
                first = oi == 0
                last = oi == len(OFFSETS) - 1
                fw = 2
                bw = 2 - dj
                if cls in ('P', 'Q'):
                    w8 = wfld
                    m8 = fld.tile([128, nh * NS], F8, tag="m8", name="m8", bufs=8)
                    nc.gpsimd.tensor_tensor(
                        out=pairap(m8[:, :], cl + 2, nh, wd),
                        in0=pairap(w8[:, :], cl + 2, nh, wd),
                        in1=pairap(dt[:, :], cl + 2, nh, wd),
                        op=Alu.mult)
                    idx = 2 * OFF8_INDEX[(di, dj)]
                    st0, st1 = M8[idx], M8[idx + 1]
                    if cls == 'P':
                        w0 = min(fw, bw)
                        stride = abs(dj)
                        for h in range(nh):
                            o = h * NS
                            nc.tensor.matmul(
                                S0[h][:, :], statpair(st0),
                                drap(w8[:, :], o + w0, stride, W),
                                start=first, stop=last, perf_mode=DR)
                            nc.tensor.matmul(
                                S1[h][:, :], statpair(st1),
                                drap(m8[:, :], o + w0, stride, W),
                                start=first, stop=last, perf_mode=DR)
                    else:  # Q: two plain fp8 matmuls each, [fw|bw] layout
                        def half(v, c0):
                            return bass.AP(tensor=v.tensor,
                                           offset=v.offset + c0,
                                           ap=[v.ap[0], [1, 128]])
                        for h in range(nh):
                            o = h * NS
                            nc.tensor.matmul(S0[h][:, :], half(st0, 0),
                                             w8[:, o + fw:o + fw + W],
                                             start=first, stop=False)
                            nc.tensor.matmul(S0[h][:, :], half(st0, 128),
                                             w8[:, o + bw:o + bw + W],
                                             start=False, stop=last)
                            nc.tensor.matmul(S1[h][:, :], half(st1, 0),
                                             m8[:, o + fw:o + fw + W],
                                             start=first, stop=False)
                            nc.tensor.matmul(S1[h][:, :], half(st1, 128),
                                             m8[:, o + bw:o + bw + W],
                                             start=False, stop=last)
                else:
                    w16 = wfld
                    m16 = fld.tile([128, nh * NS], F16, tag="m16", name="m16", bufs=8)
                    nc.vector.tensor_tensor(
                        out=pairap(m16[:, :], cl + 2, nh, wd),
                        in0=pairap(w16[:, :], cl + 2, nh, wd),
                        in1=pairap(dt[:, :], cl + 2, nh, wd),
                        op=Alu.mult)
                    if cls == 'D0':
                        base = 3 * nD + 2 * (0 if di == 1 else 1)
                        c0, c1 = M16[base], M16[base + 1]
                        for h in range(nh):
                            o = h * NS
                            nc.tensor.matmul(S0[h][:, :], c0,
                                             w16[:, o + fw:o + fw + W],
                                             start=first, stop=last)
                            nc.tensor.matmul(S1[h][:, :], c1,
                                             m16[:, o + fw:o + fw + W],
                                             start=first, stop=last)
                    else:
                        # [a*E2, a*E_{2-di}, -a*E_{2-di}]
                        base = 3 * OFFD_INDEX[(di, dj)]
                        afw, abw, nbw = (M16[base], M16[base + 1],
                                         M16[base + 2])
                        for h in range(nh):
                            o = h * NS
                            nc.tensor.matmul(S0[h][:, :], afw,
                                             w16[:, o + fw:o + fw + W],
                                             start=first, stop=False)
                            nc.tensor.matmul(S0[h][:, :], abw,
                                             w16[:, o + bw:o + bw + W],
                                             start=False, stop=last)
                            nc.tensor.matmul(S1[h][:, :], afw,
                                             m16[:, o + fw:o + fw + W],
                                             start=first, stop=False)
                            nc.tensor.matmul(S1[h][:, :], nbw,
                                             m16[:, o + bw:o + bw + W],
                                             start=False, stop=last)

            deferred = None
            ep_h = 0
            for oi, (di, dj, cls) in enumerate(OFFSETS):
                cl = min(0, -dj)
                wd = W + abs(dj)

                dt = fld.tile([128, nh * NS], F16, tag="dt", name="dt",
                              bufs=10)
                nc.vector.tensor_tensor(
                    out=pairap(dt[:, :], cl + 2, nh, wd),
                    in0=pairap(T[di][:, :], cl + dj + 2, nh, wd),
                    in1=pairap(T[0][:, :], cl + 2, nh, wd),
                    op=Alu.subtract)
                wfld = fld.tile([128, nh * NS],
                                F8 if cls in ('P', 'Q') else F16,
                                tag="w8" if cls in ('P', 'Q') else "w16",
                                name="w", bufs=8)
                nc.scalar.activation(
                    pairap(wfld[:, :], cl + 2, nh, wd),
                    pairap(dt[:, :], cl + 2, nh, wd),
                    Act.Derivative_Erf, bias=0.0, scale=SQRT50)

                if deferred is not None:
                    emit_mm(*deferred)
                deferred = (oi, dt, wfld)

                if oi == 0 and gi == 0:
                    # rest of the startup loads, after the first dt/derf
                    nc.sync.dma_start(out=SH16[:, :], in_=sh16)
                    nc.sync.dma_start(out=SH8[:, :], in_=sh8)
                    load_slab(grp, T, 1)
                    load_slab(grp, T, 2)
                if oi == 1 and gi + 1 < len(groups):
                    # prefetch next group's slabs AFTER this group's first dts
                    T_next = load_slabs(groups[gi + 1])

                # previous group's epilogue: one strip per offset slot so the
                # PSUM-drain chain spreads through the ACT/DVE queues
                if oi >= 4 and pending is not None:
                    emit_epilogue_strip(*pending, ep_h)
                    ep_h += 1
                    if ep_h == len(pending[0]):
                        pending = None
                        ep_h = 0
            emit_mm(*deferred)

            pending = (grp, T, S0, S1)
        for h in range(len(pending[0])):
            emit_epilogue_strip(*pending, h, tail=True)

    nc.compile()
    return nc


def _get_module():
    if "nc" not in _CACHE:
        _CACHE["nc"] = _build()
    return _CACHE["nc"]


def _pack_core(xc):
    """xc [C,H,W] f32 -> reflect-padded fp16 [Rpad, W+4]."""
    Rpad, _ = _strip_plan()
    xpad = np.pad(xc, ((0, 0), (2, 2), (2, 2)), mode="reflect")
    flat = xpad.reshape(C * (H + 4), W + 4)
    extra = Rpad - flat.shape[0]
    if extra > 0:
        flat = np.concatenate([flat, np.repeat(flat[-1:], extra, axis=0)],
                              axis=0)
    return np.ascontiguousarray(flat, dtype=np.float16)


def _stationaries(spatial):
    import ml_dtypes

    def E(q):  # lhsT=eye(k=-q) => out[r] = moving[r+q]
        return np.eye(128, 128, k=-q, dtype=np.float32)

    sh8_list = [None] * (2 * len(OFF8_INDEX))
    nD = len(OFFD_INDEX)
    sh16_list = [None] * (3 * nD + 4)
    for di, dj, cls in OFFSETS:
        a = float(spatial[2 + di, 2 + dj]) * DERF_SCALE
        fwm = a * E(2)
        if cls in ('P', 'Q'):
            bwm = a * E(2 - di)
            if cls == 'P' and dj > 0:  # bw window (2-dj) comes first
                s0p = np.concatenate([bwm, fwm], axis=1)
                s1p = np.concatenate([-bwm, fwm], axis=1)
            else:                      # [fw|bw] (all Q, and P with dj<0)
                s0p = np.concatenate([fwm, bwm], axis=1)
                s1p = np.concatenate([fwm, -bwm], axis=1)
            idx = 2 * OFF8_INDEX[(di, dj)]
            sh8_list[idx] = s0p
            sh8_list[idx + 1] = s1p
        elif cls == 'D0':
            base = 3 * nD + 2 * (0 if di == 1 else 1)
            sh16_list[base] = a * (E(2) + E(2 - di))
            sh16_list[base + 1] = a * (E(2) - E(2 - di))
        else:
            base = 3 * OFFD_INDEX[(di, dj)]
            sh16_list[base] = fwm
            sh16_list[base + 1] = a * E(2 - di)
            sh16_list[base + 2] = -a * E(2 - di)
    sh8 = np.concatenate(sh8_list, axis=1).astype(ml_dtypes.float8_e4m3)
    sh16 = np.concatenate(sh16_list, axis=1).astype(np.float16)
    scv = np.array([[spatial[2, 2]]], dtype=np.float32)
    return sh8, sh16, scv


def kernel(x, spatial, _trace=False):
    from concourse.bass_utils import run_bass_kernel_spmd

    x = np.asarray(x, dtype=np.float32)
    spatial = np.asarray(spatial, dtype=np.float32)
    assert x.shape == (B, C, H, W) and spatial.shape == (5, 5)
    # weight-field sharing between forward/backward taps needs symmetry
    assert np.allclose(spatial, spatial[::-1, ::-1], rtol=1e-5), \
        "kernel assumes point-symmetric spatial weights"

    sh8, sh16, scv = _stationaries(spatial)
    nc = _get_module()
    in_maps = [{"xp": _pack_core(x[b]), "sh8": sh8, "sh16": sh16, "scv": scv}
               for b in range(B)]
    res = run_bass_kernel_spmd(nc, in_maps, core_ids=list(range(B)),
                               trace=_trace)
    out = np.stack([res.results[b]["y"].astype(np.float32).reshape(C, H, W)
                    for b in range(B)])
    if _trace:
        return out, res
    return out


# revision 24
# speedup vs baseline: 1.0814x; 1.0246x over previous
"""Bilateral filter denoiser (5x5, sigma_s=2.0, sigma_r=0.1) on 8 Trainium2
NeuronCores.  Takes full inputs x (8,3,512,512) f32 + spatial (5,5) f32;
pure data parallel: one batch element per core; returns the full output.

Per-core kernel (Bass/Tile), symmetric half-offset formulation:
  For each of the 12 half offsets t=(di,dj):
      Wraw_t[g] = (2/sqrt(pi)) * exp(-50*(xp[g+t]-xp[g])^2)   [Derivative_Erf]
      mraw_t    = Wraw_t * (xp[g+t]-xp[g])
  With a_t = s_t*sqrt(pi)/2 folded into the matmul stationaries:
      S0 = s_c + sum_t a_t*(Wraw_t[g] + Wraw_t[g-t])
      S1 = sum_t a_t*(mraw_t[g] - mraw_t[g-t])
      out = x + S1/S0        (S0 >= s_c > 0: the 1e-10 clip never binds)
  Reflect padding makes the shared-weight trick exact at image borders.

Engine assignment (four-way balance DVE/ACT/Pool/PE):
  * dt on DVE (fp16 tensor_tensor, 2x mode).  W on ACT via ONE
    Derivative_Erf per offset (squaring is inside the table function;
    scale=sqrt(50) via ACT's free affine) -- no Square ops anywhere.
  * 'P' offsets (|dj|=2 ring, 5): W fp8e4 from ACT; m = W*dt on GpSimd
    in fp8; S0 and S1 each accumulate via ONE fp8 DoubleRow matmul per
    strip (2 k-tiles = fw+bw taps at col-stride |dj|, 0.5 cycles/row).
    NB k-tile stride 1 hangs the PE -- only |dj|=2 offsets can DR.
  * 'Q' offset (2,1): W+m fp8 as above but two plain fp8 matmuls each.
  * 'D'/'D0' offsets: W fp16, m on DVE fp16 (2x); fp16 matmuls (dj==0
    uses combined fw+bw matrices).
  * epilogue: S0 evacuated on ACT (Identity, +s_c bias -- replaces a
    center-tap matmul), fast-reciprocal + S1*Rc + (+x) on DVE fp16;
    y stored fp16 (host casts to fp32).
  * 13 strips of 128 padded rows in groups of 3 (+1 warmup group); PSUM
    tags rotate by global strip index so consecutive groups land in banks
    freed a group earlier (decouples the boundary drain chain and keeps
    the PE p-state ramped); field ops cover a whole group via 2-level APs;
    m+matmul emission is software-pipelined one offset behind dt/derf.
"""

import numpy as np

B, C, H, W = 8, 3, 512, 512
SQRT50 = 7.0710678118654755
DERF_SCALE = 0.8862269254527579  # sqrt(pi)/2: Derivative_Erf = 2/sqrt(pi)*exp(-x^2)

# (di, dj, cls). cls: 'P' fp8 DoubleRow, 'Q' fp8 plain, 'D' fp16, 'D0' dj==0.
# Pool computes m for P/Q (6 offsets), DVE for D/D0 (6).  fp8 m lands on the
# smallest spatial weights.  Order interleaves Pool-m and DVE-m classes.
OFFSETS = [
    (0, 1, 'D'), (1, -2, 'P'), (1, -1, 'D'), (2, 2, 'P'),
    (1, 0, 'D0'), (1, 2, 'P'), (1, 1, 'D'), (2, -2, 'P'),
    (2, 1, 'Q'), (2, 0, 'D0'), (0, 2, 'P'), (2, -1, 'D'),
]
GSZ = 3

_CACHE = {}


def _strip_plan():
    Hp = H + 4
    R = C * Hp
    strips = []
    rbase = 0
    while R - 4 - rbase > 0:
        strips.append((rbase, min(124, R - 4 - rbase)))
        rbase += 124
    return strips[-1][0] + 132, strips


# index of each fp8-class (P/Q) offset into the sh8 stationary block and of
# each D-class offset into the sh16 block
OFF8_INDEX = {}
OFFD_INDEX = {}
for _d in OFFSETS:
    if _d[2] in ('P', 'Q'):
        OFF8_INDEX[(_d[0], _d[1])] = len(OFF8_INDEX)
    elif _d[2] == 'D':
        OFFD_INDEX[(_d[0], _d[1])] = len(OFFD_INDEX)


def _build():
    from contextlib import ExitStack

    import concourse.bacc as bacc
    import concourse.bass as bass
    import concourse.tile as tile
    from concourse import mybir

    F32 = mybir.dt.float32
    F16 = mybir.dt.float16
    F8 = mybir.dt.float8e4
    Alu = mybir.AluOpType
    Act = mybir.ActivationFunctionType
    DR = mybir.MatmulPerfMode.DoubleRow

    Hp, Wp = H + 4, W + 4
    R = C * Hp
    Rpad, strips = _strip_plan()
    NS = Wp  # per-strip slot width inside group tiles

    n8 = len(OFF8_INDEX)
    nD = len(OFFD_INDEX)
    nD0 = sum(1 for o in OFFSETS if o[2] == 'D0')

    def pairap(v, col0, nh, wd):
        return bass.AP(tensor=v.tensor, offset=v.offset + col0,
                       ap=[v.ap[0], [NS, nh], [1, wd]])

    def drap(v, col0, stride, wd):
        # 2-k-tile moving operand: windows at col0 and col0+stride
        return bass.AP(tensor=v.tensor, offset=v.offset + col0,
                       ap=[v.ap[0], [stride, 2], [1, wd]])

    def statpair(v):
        return bass.AP(tensor=v.tensor, offset=v.offset,
                       ap=[v.ap[0], [128, 2], [1, 128]])

    nc = bacc.Bacc(
        "TRN2",
        target_bir_lowering=False,
        debug=False,
        enable_asserts=False,
        num_devices=B,
    )
    xp = nc.dram_tensor("xp", [Rpad, Wp], F16, kind="ExternalInput").ap()
    # stationaries packed host-side as [128, N] so each loads in ONE DMA
    # (28 separate loads serialized ~650ns each on the SP sequencer and
    # delayed kernel start by ~26us)
    sh8 = nc.dram_tensor("sh8", [128, 2 * n8 * 256], F8,
                         kind="ExternalInput").ap()
    sh16 = nc.dram_tensor("sh16", [128, (3 * nD + 2 * nD0) * 128], F16,
                          kind="ExternalInput").ap()
    scv = nc.dram_tensor("scv", [1, 1], F32, kind="ExternalInput").ap()
    y = nc.dram_tensor("y", [C * H, W], F16, kind="ExternalOutput").ap()

    with tile.TileContext(nc) as tc, ExitStack() as ctx:
        consts = ctx.enter_context(tc.tile_pool(name="consts", bufs=1))
        # consts ride the Pool (SWDGE) queue so the SP queue dispatches the
        # first slab loads immediately
        SH8 = consts.tile([128, 2 * n8 * 256], F8, name="SH8", tag="SH8")
        SH16 = consts.tile([128, (3 * nD + 2 * nD0) * 128], F16,
                           name="SH16", tag="SH16")
        M8 = [SH8[:, s * 256:(s + 1) * 256] for s in range(2 * n8)]
        M16 = [SH16[:, s * 128:(s + 1) * 128]
               for s in range(3 * nD + 2 * nD0)]
        sc = consts.tile([128, 1], F32)
        nc.gpsimd.dma_start(out=sc[:], in_=scv.to_broadcast([128, 1]))

        slabs = ctx.enter_context(tc.tile_pool(name="slabs", bufs=3))
        fld = ctx.enter_context(tc.tile_pool(name="fld", bufs=7))
        accp = ctx.enter_context(tc.tile_pool(name="accum", bufs=5))
        psum = ctx.enter_context(tc.tile_pool(name="psum", bufs=1, space="PSUM"))

        # tiny group LAST: the tail epilogue chain then covers one strip
        seq = list(enumerate(strips))
        groups = [seq[i:i + GSZ] for i in range(0, len(seq), GSZ)]

        def load_slab(grp, T, v):
            nh = len(grp)
            rb0 = grp[0][1][0]
            # one DMA per slab: strips advance by 124 rows in DRAM and
            # by NS cols in SBUF
            src_ap = bass.AP(
                tensor=xp.tensor,
                offset=xp.offset + (rb0 + v) * Wp,
                ap=[[Wp, 128], [124 * Wp, nh], [1, Wp]])
            nc.sync.dma_start(out=pairap(T[v][:, :], 0, nh, Wp), in_=src_ap)

        def load_slabs(grp):
            nh = len(grp)
            T = [slabs.tile([128, nh * NS], F16, tag=f"T{v}", name=f"T{v}")
                 for v in range(3)]
            for v in range(3):
                load_slab(grp, T, v)
            return T

        # Only T0 of group 0 loads up front: the in-order DMA queue means a
        # consumer waits for every EARLIER-emitted DMA, so all other loads
        # are emitted after the first dt/derf (see the offset loop)
        grp0 = groups[0]
        T_next = [slabs.tile([128, len(grp0) * NS], F16, tag=f"T{v}",
                             name=f"T{v}") for v in range(3)]
        load_slab(grp0, T_next, 0)

        def emit_epilogue_strip(grp, T, S0, S1, h, tail=False):
                (rbase, K) = grp[h][1]
                # S0 evacuation on ACT folds in the center tap via bias
                S0s = accp.tile([128, W], F32, tag="S0s", name="S0s")
                nc.scalar.activation(S0s[:K, :], S0[h][:K, :], Act.Identity,
                                     bias=sc[:K, :])
                Rc = accp.tile([128, W], F32, tag="Rc", name="Rc")
                nc.vector.reciprocal_approx_fast(out=Rc[:K, :],
                                                 in_=S0s[:K, :])
                tmp = accp.tile([128, W], F16, tag="tmp", name="tmp")
                nc.vector.tensor_tensor(
                    out=tmp[:K, :], in0=S1[h][:K, :], in1=Rc[:K, :],
                    op=Alu.mult)
                res = accp.tile([128, W], F16, tag="res", name="res")
                nc.vector.tensor_tensor(
                    out=res[:K, :], in0=tmp[:K, :],
                    in1=T[2][0:K, h * NS + 2:h * NS + 2 + W], op=Alu.add)

                k = 0
                while k < K:
                    g = rbase + 2 + k
                    if g < R and 2 <= (g % Hp) <= Hp - 3:
                        k1 = k
                        while k1 < K:
                            g1 = rbase + 2 + k1
                            if g1 >= R or not (2 <= (g1 % Hp) <= Hp - 3):
                                break
                            if (g1 % Hp) == 2 and k1 > k:
                                break
                            k1 += 1
                        h0 = (g // Hp) * H + (g % Hp) - 2
                        nc.sync.dma_start(out=y[h0:h0 + (k1 - k), :],
                                            in_=res[k:k1, :])
                        k = k1
                    else:
                        k += 1

        pending = None
        for gi, grp in enumerate(groups):
            nh = len(grp)
            T = T_next

            # PSUM tags rotate by GLOBAL strip index so a new group's first
            # strips land in banks freed >=1 group ago (partial decoupling
            # of the boundary drain chain)
            S0 = [psum.tile([128, W], F32, tag=f"S0p{si % 4}",
                            name=f"S0p{si % 4}") for si, _ in grp]
            S1 = [psum.tile([128, W], F32, tag=f"S1p{si % 4}",
                            name=f"S1p{si % 4}") for si, _ in grp]

            def emit_mm(oi, dt, wfld):
                # m + matmuls for offset oi, deferred one offset so the
                # in-order DVE/Pool queues never head-of-line block on the
                # ACT derf round trip
                di, dj, cls = OFFSETS[oi]
                cl = min(0, -dj)
                wd = W + abs(dj)
                first = oi == 0
                last = oi == len(OFFSETS) - 1
                fw = 2
                bw = 2 - dj
                if cls in ('P', 'Q'):
                    w8 = wfld
                    m8 = fld.tile([128, nh * NS], F8, tag="m8", name="m8", bufs=8)
                    nc.gpsimd.tensor_tensor(
                        out=pairap(m8[:, :], cl + 2, nh, wd),
                        in0=pairap(w8[:, :], cl + 2, nh, wd),
                        in1=pairap(dt[:, :], cl + 2, nh, wd),
                        op=Alu.mult)
                    idx = 2 * OFF8_INDEX[(di, dj)]
                    st0, st1 = M8[idx], M8[idx + 1]
                    if cls == 'P':
                        w0 = min(fw, bw)
                        stride = abs(dj)
                        for h in range(nh):
                            o = h * NS
                            nc.tensor.matmul(
                                S0[h][:, :], statpair(st0),
                                drap(w8[:, :], o + w0, stride, W),
                                start=first, stop=last, perf_mode=DR)
                            nc.tensor.matmul(
                                S1[h][:, :], statpair(st1),
                                drap(m8[:, :], o + w0, stride, W),
                                start=first, stop=last, perf_mode=DR)
                    else:  # Q: two plain fp8 matmuls each, [fw|bw] layout
                        def half(v, c0):
                            return bass.AP(tensor=v.tensor,
                                           offset=v.offset + c0,
                                           ap=[v.ap[0], [1, 128]])
                        for h in range(nh):
                            o = h * NS
                            nc.tensor.matmul(S0[h][:, :], half(st0, 0),
                                             w8[:, o + fw:o + fw + W],
                                             start=first, stop=False)
                            nc.tensor.matmul(S0[h][:, :], half(st0, 128),
                                             w8[:, o + bw:o + bw + W],
                                             start=False, stop=last)
                            nc.tensor.matmul(S1[h][:, :], half(st1, 0),
                                             m8[:, o + fw:o + fw + W],
                                             start=first, stop=False)
                            nc.tensor.matmul(S1[h][:, :], half(st1, 128),
                                             m8[:, o + bw:o + bw + W],
                                             start=False, stop=last)
                else:
                    w16 = wfld
                    m16 = fld.tile([128, nh * NS], F16, tag="m16", name="m16", bufs=8)
                    nc.vector.tensor_tensor(
                        out=pairap(m16[:, :], cl + 2, nh, wd),
                        in0=pairap(w16[:, :], cl + 2, nh, wd),
                        in1=pairap(dt[:, :], cl + 2, nh, wd),
                        op=Alu.mult)
                    if cls == 'D0':
                        base = 3 * nD + 2 * (0 if di == 1 else 1)
                        c0, c1 = M16[base], M16[base + 1]
                        for h in range(nh):
                            o = h * NS
                            nc.tensor.matmul(S0[h][:, :], c0,
                                             w16[:, o + fw:o + fw + W],
                                             start=first, stop=last)
                            nc.tensor.matmul(S1[h][:, :], c1,
                                             m16[:, o + fw:o + fw + W],
                                             start=first, stop=last)
                    else:
                        # [a*E2, a*E_{2-di}, -a*E_{2-di}]
                        base = 3 * OFFD_INDEX[(di, dj)]
                        afw, abw, nbw = (M16[base], M16[base + 1],
                                         M16[base + 2])
                        for h in range(nh):
                            o = h * NS
                            nc.tensor.matmul(S0[h][:, :], afw,
                                             w16[:, o + fw:o + fw + W],
                                             start=first, stop=False)
                            nc.tensor.matmul(S0[h][:, :], abw,
                                             w16[:, o + bw:o + bw + W],
                                             start=False, stop=last)
                            nc.tensor.matmul(S1[h][:, :], afw,
                                             m16[:, o + fw:o + fw + W],
                                             start=first, stop=False)
                            nc.tensor.matmul(S1[h][:, :], nbw,
                                             m16[:, o + bw:o + bw + W],
                                             start=False, stop=last)

            deferred = None
            ep_h = 0
            for oi, (di, dj, cls) in enumerate(OFFSETS):
                cl = min(0, -dj)
                wd = W + abs(dj)

                dt = fld.tile([128, nh * NS], F16, tag="dt", name="dt",
                              bufs=10)
                nc.vector.tensor_tensor(
                    out=pairap(dt[:, :], cl + 2, nh, wd),
                    in0=pairap(T[di][:, :], cl + dj + 2, nh, wd),
                    in1=pairap(T[0][:, :], cl + 2, nh, wd),
                    op=Alu.subtract)
                wfld = fld.tile([128, nh * NS],
                                F8 if cls in ('P', 'Q') else F16,
                                tag="w8" if cls in ('P', 'Q') else "w16",
                                name="w", bufs=8)
                nc.scalar.activation(
                    pairap(wfld[:, :], cl + 2, nh, wd),
                    pairap(dt[:, :], cl + 2, nh, wd),
                    Act.Derivative_Erf, bias=0.0, scale=SQRT50)

                if deferred is not None:
                    emit_mm(*deferred)
                deferred = (oi, dt, wfld)

                if oi == 0 and gi == 0:
                    # rest of the startup loads, after the first dt/derf
                    nc.sync.dma_start(out=SH16[:, :], in_=sh16)
                    nc.sync.dma_start(out=SH8[:, :], in_=sh8)
                    load_slab(grp, T, 1)
                    load_slab(grp, T, 2)
                if oi == 1 and gi + 1 < len(groups):
                    # prefetch next group's slabs AFTER this group's first dts
                    T_next = load_slabs(groups[gi + 1])

                # previous group's epilogue: one strip per offset slot so the
                # PSUM-drain chain spreads through the ACT/DVE queues
                if oi >= 4 and pending is not None:
                    emit_epilogue_strip(*pending, ep_h)
                    ep_h += 1
                    if ep_h == len(pending[0]):
                        pending = None
                        ep_h = 0
            emit_mm(*deferred)

            pending = (grp, T, S0, S1)
        for h in range(len(pending[0])):
            emit_epilogue_strip(*pending, h, tail=True)

    nc.compile()
    return nc


def _get_module():
    if "nc" not in _CACHE:
        _CACHE["nc"] = _build()
    return _CACHE["nc"]


def _pack_core(xc):
    """xc [C,H,W] f32 -> reflect-padded fp16 [Rpad, W+4]."""
    Rpad, _ = _strip_plan()
    xpad = np.pad(xc, ((0, 0), (2, 2), (2, 2)), mode="reflect")
    flat = xpad.reshape(C * (H + 4), W + 4)
    extra = Rpad - flat.shape[0]
    if extra > 0:
        flat = np.concatenate([flat, np.repeat(flat[-1:], extra, axis=0)],
                              axis=0)
    return np.ascontiguousarray(flat, dtype=np.float16)


def _stationaries(spatial):
    import ml_dtypes

    def E(q):  # lhsT=eye(k=-q) => out[r] = moving[r+q]
        return np.eye(128, 128, k=-q, dtype=np.float32)

    sh8_list = [None] * (2 * len(OFF8_INDEX))
    nD = len(OFFD_INDEX)
    sh16_list = [None] * (3 * nD + 4)
    for di, dj, cls in OFFSETS:
        a = float(spatial[2 + di, 2 + dj]) * DERF_SCALE
        fwm = a * E(2)
        if cls in ('P', 'Q'):
            bwm = a * E(2 - di)
            if cls == 'P' and dj > 0:  # bw window (2-dj) comes first
                s0p = np.concatenate([bwm, fwm], axis=1)
                s1p = np.concatenate([-bwm, fwm], axis=1)
            else:                      # [fw|bw] (all Q, and P with dj<0)
                s0p = np.concatenate([fwm, bwm], axis=1)
                s1p = np.concatenate([fwm, -bwm], axis=1)
            idx = 2 * OFF8_INDEX[(di, dj)]
            sh8_list[idx] = s0p
            sh8_list[idx + 1] = s1p
        elif cls == 'D0':
            base = 3 * nD + 2 * (0 if di == 1 else 1)
            sh16_list[base] = a * (E(2) + E(2 - di))
            sh16_list[base + 1] = a * (E(2) - E(2 - di))
        else:
            base = 3 * OFFD_INDEX[(di, dj)]
            sh16_list[base] = fwm
            sh16_list[base + 1] = a * E(2 - di)
            sh16_list[base + 2] = -a * E(2 - di)
    sh8 = np.concatenate(sh8_list, axis=1).astype(ml_dtypes.float8_e4m3)
    sh16 = np.concatenate(sh16_list, axis=1).astype(np.float16)
    scv = np.array([[spatial[2, 2]]], dtype=np.float32)
    return sh8, sh16, scv


def kernel(x, spatial, _trace=False):
    from concourse.bass_utils import run_bass_kernel_spmd

    x = np.asarray(x, dtype=np.float32)
    spatial = np.asarray(spatial, dtype=np.float32)
    assert x.shape == (B, C, H, W) and spatial.shape == (5, 5)
    # weight-field sharing between forward/backward taps needs symmetry
    assert np.allclose(spatial, spatial[::-1, ::-1], rtol=1e-5), \
        "kernel assumes point-symmetric spatial weights"

    sh8, sh16, scv = _stationaries(spatial)
    nc = _get_module()
    in_maps = [{"xp": _pack_core(x[b]), "sh8": sh8, "sh16": sh16, "scv": scv}
               for b in range(B)]
    res = run_bass_kernel_spmd(nc, in_maps, core_ids=list(range(B)),
                               trace=_trace)
    out = np.stack([res.results[b]["y"].astype(np.float32).reshape(C, H, W)
                    for b in range(B)])
    if _trace:
        return out, res
    return out
